# revision 24
# baseline (speedup 1.0000x reference)
"""Trainium2 Bass kernel for nn_CausalAttention (B=4, T=2048, d_model=1024, d_ff=2048).

Sharding: 8 cores = 4 batches x 2 query-halves. Each core owns 8 query blocks
of 128 rows, paired so causal work is balanced and the per-core program is
IDENTICAL (SPMD): the k-th owned block always computes E[k] key chunks; exact
causal masking arrives as per-core input data. Host-side input marshalling
ships operands pre-transposed and pre-cast so the device spends no PE/DVE
time on layout.

Math identities: S = q@k.T = x (Wq Wk.T) x.T  (contract d_model, skip k);
out = softmax(S) @ (x (Wv Wf)) + bf          (fuse v and the Wf projection).

Input-independent weight products and the value projection are sharded and
AllGathered (on-chip collectives):
  M  = Wq @ Wk.T   - each core computes a 128-ROW slice (fp16)
  N2 = Wv @ Wf     - each core computes a 256-col g slice (bf16)
  vf = x @ N2      - each batch-pair core computes its T-half (bf16)

The score pipeline (M, uT = xq M, S = uT.T x.T) runs in fp16: PE does fp16 at
1 cycle/row (4x fp32), and fp16's 11-bit mantissa matches what the fp32r HW
mode rounds fp32 operands to anyway (measured HW score abs err p99 ~1.7e-2 on
std-45 scores -> softmax prob err ~1%, inside the 2e-2 rel-err budget; bf16
scores would flip argmaxes). Accumulation stays fp32 in PSUM.

Layouts keep every bulk DMA partition-major with >=4KB contiguous runs per
descriptor (strided DMAs measured at ~32GB/s vs ~150GB/s): vf staging/gather
tensors are [part, tblock, col], the whole 16-chunk vf table is cached in
SBUF (64KB/part) so the P@vf accumulation never touches DRAM, and DMAs are
split across the SP and Activation HWDGE queues so semaphore-gated transfers
never stall the main load stream. The one-time exp activation-table load is
warmed at t=0.
"""

import sys
from contextlib import ExitStack

for _p in ("/opt/trn_rl_repo", "/root/.axon_site/_ro/trn_rl_repo"):
    if _p not in sys.path:
        sys.path.append(_p)

import ml_dtypes
import numpy as np

import concourse.bass as bass
import concourse.mybir as mybir
import concourse.tile as tile
from concourse import bacc
from concourse.bass_utils import run_bass_kernel_spmd
from concourse.masks import make_identity

F32 = mybir.dt.float32
F16 = mybir.dt.float16
BF16 = mybir.dt.bfloat16

B, T, C, F = 4, 2048, 1024, 2048
NB = T // 128  # 16 query/key blocks per batch
CC = C // 128  # 8 chunks of d_model
FC = F // 128  # 16 chunks of d_ff
NCORES = 8

# k-th owned block of each half; chosen so L(OWN_H[h][k]) <= E[k] for both h
# and sum(E)=72 (ideal causal: 68). E[k] = key chunks computed for block k.
OWN_H = {
    0: [15, 12, 11, 8, 7, 4, 3, 0],
    1: [14, 13, 10, 9, 6, 5, 2, 1],
}
E = [16, 14, 12, 10, 8, 6, 4, 2]
NEG = -1.0e30

ALL8 = [list(range(8))]
PAIRS = [[0, 1], [2, 3], [4, 5], [6, 7]]

_CACHE = {}


def _build_program():
    """Trace + finalize the (single, SPMD) Bass program."""
    nc = bacc.Bacc(None)

    # all operands arrive pre-transposed / pre-cast / pre-sliced from the host
    xT_ext = nc.declare_dram_parameter("xTin", [C, T], F16, isOutput=False)
    xqT_ext = nc.declare_dram_parameter("xqTin", [C, 1024], F16, isOutput=False)
    xvT_ext = nc.declare_dram_parameter("xvTb", [C, 1024], BF16, isOutput=False)
    m2_ext = nc.declare_dram_parameter("mask2", [8, 128, 256], BF16, isOutput=False)
    wqs_ext = nc.declare_dram_parameter("WqTs", [F, 128], F16, isOutput=False)
    wkT_ext = nc.declare_dram_parameter("WkTf", [F, C], F16, isOutput=False)
    wvT_ext = nc.declare_dram_parameter("WvTb", [F, C], BF16, isOutput=False)
    wfs_ext = nc.declare_dram_parameter("Wfs", [F, 256], BF16, isOutput=False)
    bf_ext = nc.declare_dram_parameter("bf", [F], F32, isOutput=False)
    out_ext = nc.declare_dram_parameter("out", [8, 128, F], F32, isOutput=True)

    with tile.TileContext(nc) as tc, ExitStack() as root:
        persist = root.enter_context(tc.tile_pool(name="persist", bufs=1))
        ps_t = root.enter_context(tc.tile_pool(name="ps_t", bufs=2, space="PSUM"))
        dram = root.enter_context(tc.tile_pool(name="dram", bufs=1, space="DRAM"))

        identbf = persist.tile([128, 128], BF16, tag="identbf")
        make_identity(nc, identbf[:, :])

        # warm the Scalar engine's exp path at t=0 with a dummy exp shaped
        # EXACTLY like the real softmax exp ([128,2048] fp32 -> bf16, bias,
        # accum, large-negative inputs). Emitted before any real tile is
        # loaded, so the garbage reads are harmless and free.
        wrm_b = persist.tile([128, 1], F32, tag="wrm_b")
        wrm_acc = persist.tile([128, 1], F32, tag="wrm_acc")
        nc.vector.tensor_copy(out=wrm_b, in_=identbf[:, :1])

        # long-lived operands (loads emitted late, where first needed)
        xT = persist.tile([128, CC, T], F16, tag="xT")  # 32KB/part
        uT = persist.tile([128, CC, 1024], F16, tag="uT")  # 16KB/part
        vfc = persist.tile([128, NB, F], BF16, tag="vfc")  # 64KB/part
        xvb = persist.tile([128, CC, 1024], BF16, tag="xvb")  # 16KB/part
        bfb = persist.tile([128, F], F32, tag="bfb")
        m2 = persist.tile([128, 8, 256], BF16, tag="m2")
        nc.scalar.activation(
            out=vfc[:, 0, :],
            in_=bfb[:, :],
            func=mybir.ActivationFunctionType.Exp,
            bias=wrm_b,
            scale=-200.0,
            accum_out=wrm_acc,
        )

        # collective buffers (DRAM); vf tensors are [part, tblock, col] so
        # every DMA touching them has 16KB contiguous runs per partition
        msl_d = dram.tile([128, C], F16, tag="msl_d")
        mall_d = dram.tile([C, C], F16, tag="mall_d", addr_space="Shared")
        n2s_d = dram.tile([128, F], BF16, tag="n2s_d")
        n2all_d = dram.tile([NCORES * 128, F], BF16, tag="n2all_d", addr_space="Shared")
        vfs_q = [
            dram.tile([128, 2, F], BF16, tag=f"vfs_q{qi}", name=f"vfs_q{qi}")
            for qi in range(4)
        ]
        vfall_q = [
            dram.tile([256, 2, F], BF16, tag=f"vfall_q{qi}", name=f"vfall_q{qi}")
            for qi in range(4)
        ]

        # ======== phase 1: N2-slice = Wv @ Wf[:, my 256 cols], AllGather ===
        with ExitStack() as ph2, nc.named_scope("p2_N2"):
            wvp = ph2.enter_context(tc.tile_pool(name="wvp", bufs=1))
            ps4 = ph2.enter_context(tc.tile_pool(name="ps4", bufs=1, space="PSUM"))
            wvT = wvp.tile([128, FC, C], BF16, tag="wvT")  # 32KB/part
            wfs = wvp.tile([128, FC, 256], BF16, tag="wfs")  # 8KB/part
            for f in range(FC):  # interleaved so f=0 operands arrive first
                nc.sync.dma_start(
                    out=wfs[:, f, :], in_=wfs_ext[f * 128 : (f + 1) * 128, :]
                )
                eng = nc.sync if f % 2 == 0 else nc.gpsimd
                eng.dma_start(
                    out=wvT[:, f, :], in_=wvT_ext[f * 128 : (f + 1) * 128, :]
                )
            for cc in range(CC):
                nc.gpsimd.dma_start(
                    out=xvb[:, cc, :], in_=xvT_ext[cc * 128 : (cc + 1) * 128, :]
                )
            n2s_sb = wvp.tile([128, CC, 256], BF16, tag="n2s_sb")  # 4KB/part
            for ah in range(2):
                nps = ps4.tile([128, 4, 512], F32, tag="nps")  # 4 banks
                for f in range(FC):
                    for a4 in range(4):
                        nc.tensor.matmul(
                            nps[:, a4, :256],
                            wvT[:, f, (ah * 4 + a4) * 128 : (ah * 4 + a4 + 1) * 128],
                            wfs[:, f, :],
                            start=(f == 0),
                            stop=(f == FC - 1),
                        )
                for a4 in range(4):
                    nc.vector.tensor_copy(
                        out=n2s_sb[:, ah * 4 + a4, :], in_=nps[:, a4, :256]
                    )
            nc.gpsimd.dma_start(out=n2s_d[:, :], in_=n2s_sb[:, :, :])
            nc.gpsimd.collective_compute(
                "AllGather",
                mybir.AluOpType.bypass,
                replica_groups=ALL8,
                ins=[n2s_d[:, :]],
                outs=[n2all_d[:, :]],
            )

        # ======== phase 2: M-rows = Wq[my 128 rows] @ WkT (fp16), AllGather
        with ExitStack() as ph1, nc.named_scope("p1_M"):
            wqp = ph1.enter_context(tc.tile_pool(name="wqp", bufs=1))
            ps1 = ph1.enter_context(tc.tile_pool(name="ps1", bufs=1, space="PSUM"))
            wqs = wqp.tile([128, FC, 128], F16, tag="wqs")  # 4KB/part
            wkT = wqp.tile([128, FC, C], F16, tag="wkT")  # 32KB/part
            for f in range(FC):
                nc.sync.dma_start(
                    out=wqs[:, f, :], in_=wqs_ext[f * 128 : (f + 1) * 128, :]
                )
                nc.sync.dma_start(
                    out=wkT[:, f, :], in_=wkT_ext[f * 128 : (f + 1) * 128, :]
                )
            msl_sb = wqp.tile([128, C], F16, tag="msl_sb")  # 2KB/part
            mps = ps1.tile([128, C], F32, tag="mps")  # 2 banks
            for f in range(FC):
                for h in range(2):
                    nc.tensor.matmul(
                        mps[:, h * 512 : (h + 1) * 512],
                        wqs[:, f, :],
                        wkT[:, f, h * 512 : (h + 1) * 512],
                        start=(f == 0),
                        stop=(f == FC - 1),
                    )
            nc.vector.tensor_copy(out=msl_sb[:, :], in_=mps[:, :])
            nc.gpsimd.dma_start(out=msl_d[:, :], in_=msl_sb[:, :])
            nc.gpsimd.collective_compute(
                "AllGather",
                mybir.AluOpType.bypass,
                replica_groups=ALL8,
                ins=[msl_d[:, :]],
                outs=[mall_d[:, :]],
            )

        # ======== phase 3: vf-half = x[my T half] @ N2 -> AllGather(pair) ==
        with ExitStack() as ph5, nc.named_scope("p4_vf"):
            n2p = ph5.enter_context(tc.tile_pool(name="n2p", bufs=1))
            st5 = ph5.enter_context(tc.tile_pool(name="st5", bufs=3))
            ps5 = ph5.enter_context(tc.tile_pool(name="ps5", bufs=2, space="PSUM"))
            n2 = n2p.tile([128, CC, F], BF16, tag="n2")  # 32KB/part
            for cc in range(CC):  # xT needed first by the p5 score matmuls
                nc.gpsimd.dma_start(
                    out=xT[:, cc, :], in_=xT_ext[cc * 128 : (cc + 1) * 128, :]
                )
            # n2all_d rows [gi 8][p 128], cols [c 8][w 256] flat
            for gi in range(NCORES):
                nc.gpsimd.dma_start(
                    out=n2[:, :, gi * 256 : (gi + 1) * 256],
                    in_=n2all_d[gi * 128 : (gi + 1) * 128, :].rearrange(
                        "p (c w) -> p c w", w=256
                    ),
                )
            for qi in range(4):  # vf in 4 pair-AllGathers of 2 t-blocks each
                vrow2 = st5.tile([128, 2, F], BF16, tag="vrow2")
                for tb2 in range(2):
                    tb = qi * 2 + tb2
                    for gh in range(2):  # halves of 1024 g-columns
                        vps = ps5.tile([128, 1024], F32, tag="vps")
                        for cc in range(CC):
                            for g in range(2):  # same xvb weights back-to-back
                                gg = gh * 2 + g
                                nc.tensor.matmul(
                                    vps[:, g * 512 : (g + 1) * 512],
                                    xvb[:, cc, tb * 128 : (tb + 1) * 128],
                                    n2[:, cc, gg * 512 : (gg + 1) * 512],
                                    start=(cc == 0),
                                    stop=(cc == CC - 1),
                                )
                        nc.vector.tensor_copy(
                            out=vrow2[:, tb2, gh * 1024 : (gh + 1) * 1024],
                            in_=vps[:, :],
                        )
                nc.gpsimd.dma_start(out=vfs_q[qi][:, :, :], in_=vrow2[:, :, :])
                nc.gpsimd.collective_compute(
                    "AllGather",
                    mybir.AluOpType.bypass,
                    replica_groups=PAIRS,
                    ins=[vfs_q[qi][:, :, :]],
                    outs=[vfall_q[qi][:, :, :]],
                )

        # ======== phase 4: uT = (xq M).T  [c2-chunk, owned-t] fp16 =========
        with ExitStack() as ph2b, nc.named_scope("p3_uT"):
            mxp = ph2b.enter_context(tc.tile_pool(name="mxp", bufs=1))
            mM = mxp.tile([128, CC, C], F16, tag="M")  # 16KB/part
            xqT = mxp.tile([128, CC, 1024], F16, tag="xqT")  # 16KB/part
            # mall_d row r = M row r -> mM[p, c1, :] = M[c1*128+p, :]
            for c1 in range(CC):
                nc.sync.dma_start(
                    out=mM[:, c1, :], in_=mall_d[c1 * 128 : (c1 + 1) * 128, :]
                )
            for cc in range(CC):
                nc.sync.dma_start(
                    out=xqT[:, cc, :], in_=xqT_ext[cc * 128 : (cc + 1) * 128, :]
                )
            for k in range(8):
                nc.sync.dma_start(out=m2[:, k, :], in_=m2_ext[k])
            bf_ap = bf_ext[:]
            nc.sync.dma_start(
                out=bfb,
                in_=bass.AP(
                    tensor=bf_ap.tensor,
                    offset=bf_ap.offset,
                    ap=[[0, 128]] + list(bf_ap.ap),
                ),
            )
            with ExitStack() as ph3:
                ps3 = ph3.enter_context(tc.tile_pool(name="ps3", bufs=2, space="PSUM"))
                for c2 in range(CC):
                    ups = [
                        ps3.tile([128, 512], F32, tag="ups", name=f"ups{c2}_{tt}")
                        for tt in range(2)
                    ]
                    for c1 in range(CC):
                        for tt in range(2):  # same mM weights back-to-back
                            nc.tensor.matmul(
                                ups[tt][:, :],
                                mM[:, c1, c2 * 128 : (c2 + 1) * 128],
                                xqT[:, c1, tt * 512 : (tt + 1) * 512],
                                start=(c1 == 0),
                                stop=(c1 == CC - 1),
                            )
                    for tt in range(2):
                        nc.vector.tensor_copy(
                            out=uT[:, c2, tt * 512 : (tt + 1) * 512], in_=ups[tt][:, :]
                        )
        # M / xqT pools closed here

        # vf cache: all 16 global key-chunks, SBUF-resident for the out
        # stages. global chunk g = half h*8 + j: gather qi = j//2 delivers
        # {2qi, 2qi+1} (member 0) and {8+2qi, 8+2qi+1} (member 1).
        for qi in range(4):
            nc.sync.dma_start(
                out=vfc[:, 2 * qi : 2 * qi + 2, :], in_=vfall_q[qi][0:128]
            )
            nc.gpsimd.dma_start(
                out=vfc[:, 8 + 2 * qi : 8 + 2 * qi + 2, :], in_=vfall_q[qi][128:256]
            )

        # ======== phase 5: attention over owned blocks =====================
        # All 8 softmax stages (scores -> softmax -> transposed probs) run
        # before any out stage: the out matmuls all depend on the LAST vf
        # gather, and the in-order PE must never stall on it mid-pipeline.
        with ExitStack() as ph6:
            st6 = ph6.enter_context(tc.tile_pool(name="st6", bufs=2))
            small = ph6.enter_context(tc.tile_pool(name="small", bufs=4))
            ps6 = ph6.enter_context(tc.tile_pool(name="ps6", bufs=1, space="PSUM"))

            # transposed probs for ALL stages: stage k owns E[k] chunks at
            # offset sum(E[:k]) (18KB/part total)
            EOFF = [sum(E[:k]) for k in range(8)]
            ptall = ph6.enter_context(tc.tile_pool(name="ptp", bufs=1)).tile(
                [128, sum(E), 128], BF16, tag="ptall"
            )

            def softmax_stage(k):
                with nc.named_scope(f"p5_sm{k}"):
                    return _softmax_stage(k)

            def _softmax_stage(k):
                """scores -> masked SBUF copy -> exp -> bf16 probs"""
                ek = E[k]
                scols = ek * 128
                s_sb = st6.tile([128, T], F32, tag="s_sb", name=f"s_sb{k}")
                for h0 in range(0, scols, 1024):
                    hw = min(1024, scols - h0)
                    sps = ps6.tile([128, 1024], F32, tag="sps", name=f"sps{k}_{h0}")
                    for c2 in range(CC):
                        for st in range(0, hw, 512):  # same uT weights twice
                            w = min(512, hw - st)
                            nc.tensor.matmul(
                                sps[:, st : st + w],
                                uT[:, c2, k * 128 : (k + 1) * 128],
                                xT[:, c2, h0 + st : h0 + st + w],
                                start=(c2 == 0),
                                stop=(c2 == CC - 1),
                            )
                    m0 = scols - 256  # mask window start
                    plain = min(hw, max(0, m0 - h0))
                    if plain > 0:
                        nc.vector.tensor_copy(
                            out=s_sb[:, h0 : h0 + plain], in_=sps[:, :plain]
                        )
                    if plain < hw:
                        nc.vector.tensor_add(
                            s_sb[:, h0 + plain : h0 + hw],
                            sps[:, plain:hw],
                            m2[:, k, h0 + plain - m0 : h0 + hw - m0],
                        )
                negmax = small.tile(
                    [128, 1], F32, tag="negmax", name=f"negmax{k}", bufs=6
                )
                nc.vector.tensor_reduce(
                    out=negmax,
                    in_=s_sb[:, :scols],
                    axis=mybir.AxisListType.X,
                    op=mybir.AluOpType.max,
                    negate=True,
                )
                psb = st6.tile([128, T], BF16, tag="psb", name=f"psb{k}", bufs=3)
                rsum = small.tile([128, 1], F32, tag="rsum", name=f"rsum{k}", bufs=6)
                nc.scalar.activation(
                    out=psb[:, :scols],
                    in_=s_sb[:, :scols],
                    func=mybir.ActivationFunctionType.Exp,
                    bias=negmax,
                    scale=1.0,
                    accum_out=rsum,
                )
                rinv = small.tile([128, 1], F32, tag="rinv", name=f"rinv{k}", bufs=6)
                nc.vector.reciprocal(out=rinv, in_=rsum)
                return psb, rinv

            def transpose_stage(k, psb):
                with nc.named_scope(f"p5_tr{k}"):
                    for sc in range(E[k]):
                        pt = ps_t.tile(
                            [128, 128], BF16, tag="pt", name=f"pt{k}_{sc}"
                        )
                        nc.tensor.transpose(
                            pt[:, :], psb[:, sc * 128 : (sc + 1) * 128], identbf[:, :]
                        )
                        nc.vector.tensor_copy(
                            out=ptall[:, EOFF[k] + sc, :], in_=pt[:, :]
                        )

            def _chunk_order(ek):
                """visit chunks in vf-gather landing order (q0..q3)"""
                return [
                    sc
                    for qi in range(4)
                    for sc in (2 * qi, 2 * qi + 1, 8 + 2 * qi, 9 + 2 * qi)
                    if sc < ek
                ]

            def out_stage(k, rinv):
                with nc.named_scope(f"p5_out{k}"):
                    return _out_stage(k, rinv)

            def _out_stage(k, rinv):
                """out = P @ vf (accum over s-chunks), epilogue"""
                ek = E[k]
                order = _chunk_order(ek)
                ops = [
                    ps6.tile([128, 1024], F32, tag=f"ops{h}", name=f"ops{k}_{h}")
                    for h in range(2)
                ]  # 2+2 banks; halves release independently for the epilogue
                for oi, sc in enumerate(order):
                    for g in range(4):
                        nc.tensor.matmul(
                            ops[g // 2][:, (g % 2) * 512 : (g % 2 + 1) * 512],
                            ptall[:, EOFF[k] + sc, :],
                            vfc[:, sc, g * 512 : (g + 1) * 512],
                            start=(oi == 0),
                            stop=(oi == ek - 1),
                        )
                orow = st6.tile([128, F], F32, tag="orow", name=f"orow{k}", bufs=2)
                for h in range(2):
                    nc.vector.scalar_tensor_tensor(
                        out=orow[:, h * 1024 : (h + 1) * 1024],
                        in0=ops[h],
                        scalar=rinv,
                        in1=bfb[:, h * 1024 : (h + 1) * 1024],
                        op0=mybir.AluOpType.mult,
                        op1=mybir.AluOpType.add,
                    )
                nc.sync.dma_start(out=out_ext[k], in_=orow)

            rinvs = {}
            prev_psb = None
            for k in range(8):
                psb, rinv = softmax_stage(k)
                rinvs[k] = rinv
                if prev_psb is not None:  # T_{k-1} after sm_k's scores: the
                    transpose_stage(k - 1, prev_psb)  # exp latency is hidden
                prev_psb = psb
            transpose_stage(7, prev_psb)
            for k in range(8):
                out_stage(k, rinvs[k])

    nc.finalize()
    return nc


def _get_program():
    if "nc" not in _CACHE:
        _CACHE["nc"] = _build_program()
    return _CACHE["nc"]


def _make_in_maps(x, Wq, Wk, Wv, Wf, bf):
    x = np.ascontiguousarray(x, dtype=np.float32)
    WqT = np.ascontiguousarray(np.asarray(Wq, dtype=np.float32).T)
    WkT16 = np.ascontiguousarray(np.asarray(Wk, dtype=np.float32).T).astype(np.float16)
    WvTb = np.ascontiguousarray(np.asarray(Wv, dtype=np.float32).T).astype(
        ml_dtypes.bfloat16
    )
    Wfb = np.asarray(Wf, dtype=np.float32).astype(ml_dtypes.bfloat16)
    bf = np.ascontiguousarray(bf, dtype=np.float32)
    in_maps = []
    for core in range(NCORES):
        b, h = core // 2, core % 2
        own = OWN_H[h]
        xb = x[b]
        xq = np.concatenate([xb[blk * 128 : (blk + 1) * 128] for blk in own], axis=0)
        mask2 = np.zeros((8, 128, 256), dtype=np.float32)  # cast below
        for k, blk in enumerate(own):
            s0 = (E[k] - 2) * 128  # global key index of mask window start
            s = s0 + np.arange(256)[None, :]
            t = blk * 128 + np.arange(128)[:, None]
            mask2[k] = np.where(s <= t, 0.0, NEG).astype(np.float32)
        xvTb = (
            np.ascontiguousarray(xb[h * 1024 : (h + 1) * 1024].T)
            .astype(ml_dtypes.bfloat16)
        )
        in_maps.append(
            {
                "xTin": np.ascontiguousarray(xb.T).astype(np.float16),
                "xqTin": np.ascontiguousarray(xq.T).astype(np.float16),
                "xvTb": xvTb,
                "mask2": mask2.astype(ml_dtypes.bfloat16),
                "WqTs": np.ascontiguousarray(
                    WqT[:, core * 128 : (core + 1) * 128]
                ).astype(np.float16),
                "WkTf": WkT16,
                "WvTb": WvTb,
                "Wfs": np.ascontiguousarray(Wfb[:, core * 256 : (core + 1) * 256]),
                "bf": bf,
            }
        )
    return in_maps


def run_on_hw(inputs, trace=False, trace_cores=None):
    nc = _get_program()
    in_maps = _make_in_maps(**inputs)
    res = run_bass_kernel_spmd(
        nc, in_maps, list(range(NCORES)), trace=trace, trace_cores=trace_cores
    )
    out = np.empty((B, T, F), dtype=np.float32)
    for core in range(NCORES):
        b, h = core // 2, core % 2
        own = OWN_H[h]
        o = res.results[core]["out"]  # [8, 128, F]
        for k, blk in enumerate(own):
            out[b, blk * 128 : (blk + 1) * 128, :] = o[k]
    return out, res


def kernel(x, Wq, Wk, Wv, Wf, bf):
    out, _ = run_on_hw(dict(x=x, Wq=Wq, Wk=Wk, Wv=Wv, Wf=Wf, bf=bf))
    return out
